# revision 3
# baseline (speedup 1.0000x reference)
"""DIEN (GRU + AUGRU scan) Trainium2 Bass kernel — v2 (fp16 PE, restructured chain).

Strategy
--------
Data-parallel over batch: B=256 split 8 ways (32 per core); weights replicated;
the T=200 scan is sequential per core.

Key wins over v1 (fp32 baseline):
  * fp16 matmul operands: the PE streams 1 cycle/row for fp16 vs 4 for fp32.
    PSUM accumulation stays fp32.
  * Biases injected into PSUM by one K=4 matmul per cell-step (selector
    stationary), removing all bias adds from the recurrence chain.
  * Sequence pre-transposed host-side into the stationary (T) layout; the gi
    input projection runs one step ahead of the recurrence in the PE queue.
  * Per cell-step one PSUM bank [rz(256) | gi_n(128) | gh_n(128)], 3-deep
    rotation; rz matmuls emitted before n matmuls so the sigmoid starts early.
  * Blend restructured: zb=1-z and zh=z*h off-chain on GPSIMD; critical path
    after tanh is nb=zb*n, h=nb+zh, blocktranspose.

Layouts (per core, batch b in 0..31, hidden h = 128*c + 32*m + jr):
  row layout : tile[32*c + b, 32*m + jr]   (gates, states)
  T layout   : tileT[32*c + jr, 32*m + b]  (stationary; 32x32 block transpose
               of row layout). K-tile k of a GEMM is tileT[:, 32k:32k+32],
               contracting hidden dims {128c + 32k + jr}.
Weights are pre-arranged host-side to match (same as v1 _arrange_w).
"""

import os
import sys

import numpy as np

for _p in ("/opt/trn_rl_repo", "/root/.axon_site/_ro/trn_rl_repo"):
    if os.path.isdir(_p) and _p not in sys.path:
        sys.path.append(_p)

B, T, H = 256, 200, 512
N_CORES = 8
BL = B // N_CORES  # 32

_CACHE = {}


# ---------------------------------------------------------------------------
# Host-side weight preparation (pure numpy, exact rearrangements)
# ---------------------------------------------------------------------------

def _arrange_w(W, gate_order=(0, 1, 2)):
    """[3H, H] (out, in) -> [128, 4, 1536] K-tile-arranged weight blocks.

    Block m, partition p = 32*c_in + jr holds input dim h_in = 128*c_in + 32*m + jr.
    Free index f = c_out*384 + g*128 + j maps output col gate_order[g]*512 +
    c_out*128 + j. Input-side weights use gate_order (2,0,1) = [n|r|z] so the
    input matmul's 384 cols land at psum [gin|rz]; hidden-side uses (0,1,2) =
    [r|z|n] so they land at [rz|ghn].
    """
    A = W.T.reshape(4, 4, 32, 3 * H)                # [c_in, m, jr, out]
    A = A.transpose(1, 0, 2, 3).reshape(4, 128, 3 * H)
    A = A.reshape(4, 128, 3, 4, 128)[:, :, list(gate_order)]
    A = A.transpose(0, 1, 3, 2, 4).reshape(4, 128, 3 * H)
    A = A.transpose(1, 0, 2)                        # [p, m, out] for contiguous DMA
    return np.ascontiguousarray(A, dtype=np.float16)


def _bias4(b_rz, b_in, b_hn):
    """Chunked bias rows -> [4, 512] tile: row c = [gin(128) | rz(256) | ghn(128)].

    b_rz: [1024] r|z bias (input+hidden summed), b_in: [512] input-side n bias,
    b_hn: [512] hidden-side n bias.
    """
    out = np.zeros((4, 512), np.float32)
    for c in range(4):
        out[c, 0:128] = b_in[c * 128:(c + 1) * 128]
        out[c, 128:256] = b_rz[c * 128:(c + 1) * 128]
        out[c, 256:384] = b_rz[512 + c * 128:512 + (c + 1) * 128]
        out[c, 384:512] = b_hn[c * 128:(c + 1) * 128]
    return out.astype(np.float16)


def _arrange_seq(seq_core):
    """[BL, T, H] -> [T, 128, 128] fp16 in T layout: dst[t, 32c+jr, 32m+b]."""
    s = seq_core.reshape(BL, T, 4, 4, 32)           # [b, t, c, m, jr]
    s = s.transpose(1, 2, 4, 3, 0)                  # [t, c, jr, m, b]
    return np.ascontiguousarray(s.reshape(T, 128, 128), dtype=np.float16)


# ---------------------------------------------------------------------------
# Bass program
# ---------------------------------------------------------------------------

def _build_program(n_steps=T, reps=1):
    import concourse.bacc as bacc
    import concourse.tile as tile
    from concourse import mybir
    from contextlib import ExitStack

    F32 = mybir.dt.float32
    F16 = mybir.dt.float16
    Sigmoid = mybir.ActivationFunctionType.Sigmoid
    Tanh = mybir.ActivationFunctionType.Tanh
    MUL = mybir.AluOpType.mult
    ADD = mybir.AluOpType.add

    nc = bacc.Bacc("TRN2", target_bir_lowering=False, debug=False)

    seq = nc.declare_dram_parameter("seq", [n_steps, 128, 128], F16, isOutput=False)
    w_dram = {
        name: nc.declare_dram_parameter(name, [128, 4, 3 * H], F16, isOutput=False)
        for name in ("wgi", "wgh", "wai", "wah")
    }
    bg_dram = nc.declare_dram_parameter("bg4", [1, 2048], F16, isOutput=False)
    ba_dram = nc.declare_dram_parameter("ba4", [1, 2048], F16, isOutput=False)
    out = nc.declare_dram_parameter("out", [BL, H], F32, isOutput=True)

    with tile.TileContext(nc) as tc, ExitStack() as ctx:
        wpool = ctx.enter_context(tc.tile_pool(name="weights", bufs=1))
        xt_pool = ctx.enter_context(tc.tile_pool(name="xt", bufs=4))
        st_pool = ctx.enter_context(tc.tile_pool(name="states", bufs=2))
        tmp_pool = ctx.enter_context(tc.tile_pool(name="tmps", bufs=2))
        psum_pool = ctx.enter_context(tc.tile_pool(name="psum", bufs=4, space="PSUM"))

        # --- constants: weights + biases + selector ---
        wsb = {}
        for name, drm in w_dram.items():
            t = wpool.tile([128, 4 * 3 * H], F16, tag=name, name=name)
            nc.sync.dma_start(out=t, in_=drm[:].rearrange("p m f -> p (m f)"))
            wsb[name] = t
        bg4 = wpool.tile([1, 2048], F16, tag="bg4")
        nc.sync.dma_start(out=bg4, in_=bg_dram[:])
        ba4 = wpool.tile([1, 2048], F16, tag="ba4")
        nc.sync.dma_start(out=ba4, in_=ba_dram[:])
        ones1 = wpool.tile([1, 32], F16, tag="ones1")
        nc.vector.memset(ones1, 1.0)
        zrow = wpool.tile([128, 128], F16, tag="zrow")
        nc.vector.memset(zrow, 0.0)

        def dma_x(t_):
            xt = xt_pool.tile([128, 128], F16, tag="xt", name="xt")
            nc.sync.dma_start(out=xt, in_=seq[t_, :, :])
            return xt

        def mm_bias(p, brow):
            # p[32c+b, :] = bias row c: four col-tiled K=1 ones-matmuls that
            # stay in the 32-col-tile flow (a full-width matmul here costs a
            # PE pipeline flush).
            for c in range(4):
                nc.tensor.matmul(
                    out=p[32 * c:32 * c + 32, 0:512],
                    lhsT=ones1[:, 0:32], rhs=brow[:, 512 * c:512 * c + 512],
                    start=True, stop=False, skip_group_check=True,
                    tile_position=(0, 32 * c),
                )

        def mm_group(p, srcT, w, lo, hi, stop_last):
            # p[32c+b, lo:hi] += src @ W, one N=(hi-lo) matmul per (k, c)
            for k in range(4):
                for c in range(4):
                    base = k * 1536 + 384 * c
                    nc.tensor.matmul(
                        out=p[32 * c:32 * c + 32, lo:hi],
                        lhsT=srcT[:, 32 * k:32 * k + 32],
                        rhs=w[:, base:base + 384],
                        start=False, stop=(stop_last and k == 3),
                        skip_group_check=True,
                        tile_position=(0, 32 * c),
                    )

        def mm_in(p, srcT, w):
            # input side, [n|r|z] weight order -> psum [gin | rz]
            mm_group(p, srcT, w, 0, 384, False)

        def mm_hh(p, statT, w):
            # hidden side, [r|z|n] weight order -> psum [rz (accum) | ghn]
            mm_group(p, statT, w, 128, 512, True)

        def cell(p, h_prev, tag, blend_dve):
            """Gate nonlinearity + state blend. Returns (h_row, hT) fp16."""
            rz = tmp_pool.tile([128, 256], F16, tag=tag + "rz", name="rz")
            nc.scalar.activation(rz, p[:, 128:384], Sigmoid)
            r = rz[:, 0:128]
            z = rz[:, 128:256]
            # off-chain: zb = 1 - z ; zh = z * h_prev   (GPSIMD, SBUF only)
            zb = tmp_pool.tile([128, 128], F16, tag=tag + "zb", name="zb")
            nc.gpsimd.tensor_scalar(zb, z, -1.0, 1.0, MUL, ADD)
            zh = tmp_pool.tile([128, 128], F16, tag=tag + "zh", name="zh")
            nc.gpsimd.tensor_mul(zh, z, h_prev)
            # chain: v = r * gh_n ; q = v + gi_n ; n = tanh(q)
            v = tmp_pool.tile([128, 128], F16, tag=tag + "v", name="v")
            nc.vector.tensor_mul(v, r, p[:, 384:512])
            q = tmp_pool.tile([128, 128], F16, tag=tag + "q", name="q")
            nc.vector.tensor_add(q, v, p[:, 0:128])
            n = tmp_pool.tile([128, 128], F16, tag=tag + "n", name="n")
            nc.scalar.activation(n, q, Tanh)
            eng = nc.vector if blend_dve else nc.gpsimd
            nb = tmp_pool.tile([128, 128], F16, tag=tag + "nb", name="nb")
            eng.tensor_mul(nb, zb, n)
            h_new = st_pool.tile([128, 128], F16, tag=tag + "h", name="h_new")
            eng.tensor_add(h_new, nb, zh)
            hT = st_pool.tile([128, 128], F16, tag=tag + "hT", name="hT")
            nc.vector.transpose(hT, h_new)
            return h_new, hT

        a_fin = None
        for _rep in range(reps):
            # --- prologue: prefetch x, bias+gi for step 0 ---
            xts = {0: dma_x(0), 1: dma_x(1)}
            pg = {}
            pa = {}
            pg[0] = psum_pool.tile([128, 512], F32, tag="pg", name="pg")
            pa[0] = psum_pool.tile([128, 512], F32, tag="pa", name="pa")
            mm_bias(pg[0], bg4)
            mm_in(pg[0], xts[0], wsb["wgi"])
            mm_bias(pa[0], ba4)

            g_hist = {-1: (zrow, zrow)}
            a_hist = {-1: (zrow, zrow)}

            for i in range(n_steps + 1):
                # GRU cell for step i
                if i < n_steps:
                    if i > 0:
                        mm_hh(pg[i], g_hist[i - 1][1], wsb["wgh"])
                    g_hist[i] = cell(pg[i], g_hist[i - 1][0], "g", True)
                # AUGRU matmuls + cell for step i-1
                if i >= 1:
                    j = i - 1
                    mm_in(pa[j], g_hist[j][1], wsb["wai"])
                    if j > 0:
                        mm_hh(pa[j], a_hist[j - 1][1], wsb["wah"])
                    a_hist[j] = cell(pa[j], a_hist[j - 1][0], "a", False)
                # lookahead: x DMA, gi for step i+1, bias for next psums
                if i + 2 < n_steps:
                    xts[i + 2] = dma_x(i + 2)
                if i + 1 < n_steps:
                    pg[i + 1] = psum_pool.tile([128, 512], F32, tag="pg", name="pg")
                    pa[i + 1] = psum_pool.tile([128, 512], F32, tag="pa", name="pa")
                    mm_bias(pg[i + 1], bg4)
                    mm_in(pg[i + 1], xts[i + 1], wsb["wgi"])
                    mm_bias(pa[i + 1], ba4)
                # drop stale refs
                for d, idx in ((g_hist, i - 2), (a_hist, i - 3), (pg, i - 1),
                               (pa, i - 2), (xts, i)):
                    d.pop(idx, None)

            # --- epilogue: final AUGRU state -> fp32 -> DRAM ---
            a_fin = a_hist[n_steps - 1][0]
        ofin = wpool.tile([128, 128], F32, tag="ofin")
        nc.scalar.copy(ofin, a_fin)

        import concourse.bass as bass_mod
        out_ap = bass_mod.AP(
            tensor=out[:].tensor,
            offset=0,
            ap=[[128, 4], [H, BL], [1, 128]],
        )
        nc.sync.dma_start(out=out_ap, in_=ofin)

    nc.compile()
    return nc


def _get_program(n_steps=T, reps=1):
    key = ("prog", n_steps, reps)
    if key not in _CACHE:
        _CACHE[key] = _build_program(n_steps, reps)
    return _CACHE[key]


# ---------------------------------------------------------------------------
# Entry point
# ---------------------------------------------------------------------------

def _make_in_maps(inputs):
    seq_emb = np.ascontiguousarray(np.asarray(inputs["seq_emb"], np.float32))
    augru_Wih = np.asarray(inputs["augru_Wih"])
    A1 = augru_Wih[:, :H]
    A2 = augru_Wih[:, H:]
    w_fused = (A1 + A2 @ np.asarray(inputs["v_W"])).astype(np.float32)
    b_ai = (np.asarray(inputs["augru_bih"]) + A2 @ np.asarray(inputs["v_b"])).astype(np.float32)
    b_ah = np.asarray(inputs["augru_bhh"], np.float32)
    gru_bih = np.asarray(inputs["gru_bih"], dtype=np.float32)
    gru_bhh = np.asarray(inputs["gru_bhh"], dtype=np.float32)

    bg_sum = gru_bih + gru_bhh
    ba_sum = b_ai + b_ah
    consts = {
        "wgi": _arrange_w(np.asarray(inputs["gru_Wih"]), (2, 0, 1)),
        "wgh": _arrange_w(np.asarray(inputs["gru_Whh"])),
        "wai": _arrange_w(w_fused, (2, 0, 1)),
        "wah": _arrange_w(np.asarray(inputs["augru_Whh"])),
        "bg4": _bias4(bg_sum[:1024], gru_bih[1024:], gru_bhh[1024:]).reshape(1, 2048),
        "ba4": _bias4(ba_sum[:1024], b_ai[1024:], b_ah[1024:]).reshape(1, 2048),
    }
    return [
        {"seq": _arrange_seq(seq_emb[c * BL:(c + 1) * BL]), **consts}
        for c in range(N_CORES)
    ]


def _prep_and_run(trace=False, **inputs):
    from concourse.bass_utils import run_bass_kernel_spmd

    in_maps = _make_in_maps(inputs)
    nc = _get_program()
    res = run_bass_kernel_spmd(nc, in_maps, list(range(N_CORES)), trace=trace)
    out = np.concatenate([res.results[c]["out"] for c in range(N_CORES)], axis=0)
    return out.astype(np.float32), res


def kernel(**inputs):
    out, _ = _prep_and_run(**inputs)
    return out


def kernel_traced(**inputs):
    """Like kernel() but profiles the run; returns (output, BassKernelResults)."""
    return _prep_and_run(**inputs, trace=True)


if __name__ == "__main__":
    rng = np.random.default_rng(0)
    ins = {
        "seq_emb": rng.standard_normal((B, T, H), dtype=np.float32),
        "target_emb": rng.standard_normal((B, H), dtype=np.float32),
        "gru_Wih": rng.standard_normal((3 * H, H), dtype=np.float32) * 0.04,
        "gru_Whh": rng.standard_normal((3 * H, H), dtype=np.float32) * 0.04,
        "gru_bih": rng.standard_normal(3 * H).astype(np.float32) * 0.04,
        "gru_bhh": rng.standard_normal(3 * H).astype(np.float32) * 0.04,
        "q_W": rng.standard_normal((H, H), dtype=np.float32) * 0.04,
        "q_b": rng.standard_normal(H).astype(np.float32) * 0.04,
        "k_W": rng.standard_normal((H, H), dtype=np.float32) * 0.04,
        "k_b": rng.standard_normal(H).astype(np.float32) * 0.04,
        "v_W": rng.standard_normal((H, H), dtype=np.float32) * 0.04,
        "v_b": rng.standard_normal(H).astype(np.float32) * 0.04,
        "augru_Wih": rng.standard_normal((3 * H, 2 * H), dtype=np.float32) * 0.04,
        "augru_Whh": rng.standard_normal((3 * H, H), dtype=np.float32) * 0.04,
        "augru_bih": rng.standard_normal(3 * H).astype(np.float32) * 0.04,
        "augru_bhh": rng.standard_normal(3 * H).astype(np.float32) * 0.04,
    }
    o = kernel(**ins)
    print("kernel output", o.shape, o.dtype, float(np.abs(o).max()))
